# revision 1
# baseline (speedup 1.0000x reference)
"""Trainium2 Bass kernel for additive (Bahdanau) attention.

  context[b] = sum_t softmax_t( v . tanh(We @ enc[b,t] + Wd @ dec[b] + bias) ) * enc[b,t]

Shapes (hardcoded): enc_out [64, 2048, 1024] f32, dec_state [64, 1024] f32,
W_weight [1024, 2048], W_bias [1024], v_weight [1, 1024].  Output [64, 1024].

Sharding: data-parallel over batch across 8 NeuronCores (8 batches/core).
Host prep: We^T relayout, v replication, and the tiny bias term
z = Wd @ dec + W_bias (0.05% of FLOPs) replicated to 128 partitions.

Per-core, one global software pipeline over 128 row-tiles [128t x 1024e].
All matmuls use float32r (TF32-class, ~11 mantissa bits, 1 cycle/row on the
PE vs 4 for plain fp32; fp32 accumulate in PSUM).  PE stream per step k:
  transpose(k)  8x 128x128 is_transpose matmuls of the enc tile -> PSUM
  ctx(k-2)      2x N=512 matmuls: ctx_unnorm += exp(s)^T @ X
  proj(k-1)     16x N=512 matmuls: proj = X @ We^T (K over 8 e-tiles)
so the PSUM->SBUF transpose copies (split ACT/DVE) and the DVE/ACT epilogue
(z-add, tanh, fused v-mult+reduce via scalar_tensor_tensor, exp) of one step
overlap the next step's PE work.  Softmax needs no max-subtraction
(|scores| <= sum|v| <= 32, exp safe in fp32), so exp weights are final and
ctx_unnorm accumulates across all 16 t-tiles in PSUM; one reciprocal scale
per batch normalizes.  Measured: ~675 us/core, rel err ~1.5e-4 (f32r rounding).
"""

import os
import sys

sys.path.insert(0, "/opt/trn_rl_repo")

from contextlib import ExitStack

import numpy as np

import concourse.bass as bass
import concourse.tile as tile
from concourse import bacc, mybir
from concourse.bass import ts
from concourse.bass_utils import run_bass_kernel_spmd

F32 = mybir.dt.float32
F32R = mybir.dt.float32r

B, T, E, D = 64, 2048, 1024, 1024
CORES = 8
BL = B // CORES           # batches per core
P = 128                   # partitions
TT = T // P               # t-tiles per batch (16)
ET = E // P               # e-tiles (K tiles) per row-tile (8)
CTX_LAG = 2               # t-tiles of lag before emitting ctx matmuls


def _build_kernel(bl=BL, t_tiles=TT):
    nc = bacc.Bacc(
        "TRN2",
        target_bir_lowering=False,
        debug=False,
        num_devices=CORES,
    )
    t_rows = t_tiles * P

    enc = nc.declare_dram_parameter("enc", [bl, t_rows, E], F32R, isOutput=False)
    # We^T prearranged to [128, ET*1024]: block j holds We.T[j*128:(j+1)*128, :]
    wet = nc.declare_dram_parameter("wet", [P, ET * D], F32R, isOutput=False)
    # z = Wd @ dec[b] + bias, computed host-side, replicated across 128 partitions
    zrepp = nc.declare_dram_parameter("zrepp", [P, bl, D], F32, isOutput=False)
    vrep = nc.declare_dram_parameter("vrep", [P, D], F32, isOutput=False)
    ident = nc.declare_dram_parameter("ident", [P, P], F32R, isOutput=False)
    onesc = nc.declare_dram_parameter("onesc", [P, 1], F32, isOutput=False)
    out = nc.declare_dram_parameter("ctx_out", [bl, E], F32, isOutput=True)

    with tile.TileContext(nc) as tc, ExitStack() as ctx:
        const = ctx.enter_context(tc.tile_pool(name="const", bufs=1))
        xpool = ctx.enter_context(tc.tile_pool(name="x", bufs=4 + CTX_LAG))
        xtpool = ctx.enter_context(tc.tile_pool(name="xt", bufs=3))
        epool = ctx.enter_context(tc.tile_pool(name="e", bufs=3))
        small = ctx.enter_context(tc.tile_pool(name="small", bufs=2))

        ps_xt = ctx.enter_context(tc.tile_pool(name="ps_xt", bufs=2, space="PSUM"))
        ps_proj = ctx.enter_context(tc.tile_pool(name="ps_proj", bufs=2, space="PSUM"))
        ps_ctx = ctx.enter_context(tc.tile_pool(name="ps_ctx", bufs=2, space="PSUM"))

        # ---- resident constants. Order matters: the first enc tiles, identity and
        # zrep row 0 must not queue behind the 4MB of We^T on the sync queue.
        ident_sb = const.tile([P, P], F32R)
        nc.sync.dma_start(ident_sb[:], ident[:])
        n_pre = min(4, t_tiles)
        x_pre = [
            xpool.tile([P, E], F32R, tag="x", name=f"x_pre{i}") for i in range(n_pre)
        ]
        for i in range(n_pre):
            nc.sync.dma_start(x_pre[i][:], enc[0, ts(i, P), :])
        zrep_sb = const.tile([P, bl, D], F32)
        nc.sync.dma_start(zrep_sb[:, 0, :], zrepp[:, 0, :])
        vrep_sb = const.tile([P, D], F32)
        nc.sync.dma_start(vrep_sb[:], vrep[:])
        # weights as per-block tiles so consumers wait per 512KB block, not 4MB
        wet_t = []
        for j in range(ET):
            wj = const.tile([P, D], F32R, name=f"wet{j}")
            nc.sync.dma_start(wj[:], wet[:, j * D : (j + 1) * D])
            wet_t.append(wj)
        onesc_sb = const.tile([P, 1], F32)
        nc.sync.dma_start(onesc_sb[:], onesc[:])

        # ---- main loop: one global software pipeline over all (batch, t-tile) --
        total = bl * t_tiles
        state = {}

        def get_state(b):
            if b not in state:
                state[b] = dict(
                    s_all=small.tile([P, t_tiles], F32, tag="s", name=f"s_all_{b}"),
                    p_all=small.tile([P, t_tiles], F32, tag="p", name=f"p_all_{b}"),
                    ctx0=ps_ctx.tile([1, 512], F32, tag="ps_ctx", name=f"ctx0_{b}"),
                    ctx1=ps_ctx.tile([1, 512], F32, tag="ps_ctx", name=f"ctx1_{b}"),
                    x_tiles=[None] * t_tiles,
                    xt_sbs=[None] * t_tiles,
                    p_r_cols=[None] * t_tiles,
                )
            return state[b]

        def emit_load_transpose(b, i):
            st = get_state(b)
            if b == 0 and i < n_pre:
                x_tile = x_pre[i]
            else:
                x_tile = xpool.tile([P, E], F32R, tag="x")
                nc.sync.dma_start(x_tile[:], enc[b, ts(i, P), :])
            st["x_tiles"][i] = x_tile
            # transpose X tile 128x128-blockwise:
            # xt[e_loc, j*128 + t] = x[t, j*128+e_loc]; copies split ACT/DVE
            xt_sb = xtpool.tile([P, E], F32R, tag="xt_sb")
            st["xt_sbs"][i] = xt_sb
            for g in range(2):
                xt_ps = ps_xt.tile([P, 512], F32R, tag="ps_xt")
                for j4 in range(4):
                    j = g * 4 + j4
                    nc.tensor.transpose(
                        xt_ps[:, j4 * P : (j4 + 1) * P],
                        x_tile[:, j * P : (j + 1) * P],
                        ident_sb[:],
                    )
                if g == 0:
                    nc.scalar.copy(xt_sb[:, g * 512 : (g + 1) * 512], xt_ps[:])
                else:
                    nc.vector.tensor_copy(xt_sb[:, g * 512 : (g + 1) * 512], xt_ps[:])

        def emit_proj_epilogue(b, i):
            # proj[t, d] = sum_e x[t, e] * WeT[e, d]   (K over 8 e-tiles)
            st = get_state(b)
            xt_sb = st["xt_sbs"][i]
            proj_ps = ps_proj.tile([P, D], F32, tag="ps_proj")
            for j in range(ET):
                lhs = xt_sb[:, j * P : (j + 1) * P]
                nc.tensor.matmul(
                    proj_ps[:, 0:512], lhs, wet_t[j][:, 0:512],
                    start=(j == 0), stop=(j == ET - 1),
                )
                nc.tensor.matmul(
                    proj_ps[:, 512:D], lhs, wet_t[j][:, 512:D],
                    start=(j == 0), stop=(j == ET - 1),
                )
            # energy = tanh(proj + z); s = sum_d energy * v
            e_sb = epool.tile([P, D], F32, tag="e")
            nc.vector.tensor_add(e_sb[:], proj_ps[:], zrep_sb[:, b, :])
            nc.scalar.activation(e_sb[:], e_sb[:], mybir.ActivationFunctionType.Tanh)
            nc.vector.scalar_tensor_tensor(
                out=e_sb[:],
                in0=e_sb[:],
                scalar=1.0,
                in1=vrep_sb[:],
                op0=mybir.AluOpType.mult,
                op1=mybir.AluOpType.mult,
                accum_out=st["s_all"][:, i : i + 1],
            )
            nc.scalar.activation(
                st["p_all"][:, i : i + 1],
                st["s_all"][:, i : i + 1],
                mybir.ActivationFunctionType.Exp,
            )
            p_r = small.tile([P, 1], F32R, tag="pr")
            st["p_r_cols"][i] = p_r
            nc.vector.tensor_copy(p_r[:], st["p_all"][:, i : i + 1])

        def emit_ctx(b, i):
            # ctx_unnorm += p^T @ X  (contraction over the 128 t-rows)
            st = get_state(b)
            p_col = st["p_r_cols"][i][:]
            nc.tensor.matmul(
                st["ctx0"][:], p_col, st["x_tiles"][i][:, 0:512],
                start=(i == 0), stop=(i == t_tiles - 1),
            )
            nc.tensor.matmul(
                st["ctx1"][:], p_col, st["x_tiles"][i][:, 512:E],
                start=(i == 0), stop=(i == t_tiles - 1),
            )
            if i == t_tiles - 1:
                emit_batch_end(b)

        def emit_batch_end(b):
            # l = sum_t exp(s_t); ctx = ctx_unnorm / l
            st = state.pop(b)
            l_part = small.tile([P, 1], F32, tag="lp")
            nc.vector.tensor_reduce(
                l_part[:], st["p_all"][:],
                axis=mybir.AxisListType.X, op=mybir.AluOpType.add,
            )
            l_ps = ps_xt.tile([1, 1], F32, tag="ps_xt")
            nc.tensor.matmul(l_ps[:], l_part[:], onesc_sb[:])
            linv = small.tile([1, 1], F32, tag="linv")
            nc.vector.reciprocal(linv[:], l_ps[:])
            ctx_row = small.tile([1, E], F32, tag="ctxrow")
            nc.scalar.activation(
                ctx_row[:, 0:512], st["ctx0"][:],
                mybir.ActivationFunctionType.Copy, scale=linv[:],
            )
            nc.scalar.activation(
                ctx_row[:, 512:E], st["ctx1"][:],
                mybir.ActivationFunctionType.Copy, scale=linv[:],
            )
            nc.sync.dma_start(out[b : b + 1, :], ctx_row[:])

        # PE stream per step k: transp(k) -> ctx(k-2) -> proj(k-1); the psum->sbuf
        # transpose copies of step k overlap with proj(k-1) on ACT/DVE.
        for k in range(total + 2):
            if 0 < k <= bl - 1:
                nc.sync.dma_start(zrep_sb[:, k, :], zrepp[:, k, :])
            if k < total:
                emit_load_transpose(*divmod(k, t_tiles))
            if k - CTX_LAG >= 0:
                emit_ctx(*divmod(k - CTX_LAG, t_tiles))
            if k - 1 >= 0 and k - 1 < total:
                emit_proj_epilogue(*divmod(k - 1, t_tiles))

    nc.compile()
    return nc


def _prep_inputs(enc_out, dec_state, W_weight, W_bias, v_weight, bl=BL):
    """Host-side layout prep (transposes/replication + the tiny Wd@dec bias
    term, 0.05% of FLOPs) + per-core slicing."""
    enc_out = np.ascontiguousarray(enc_out, dtype=np.float32)
    dec_state = np.ascontiguousarray(dec_state, dtype=np.float32)
    W = np.asarray(W_weight, dtype=np.float32)
    wet_h = np.ascontiguousarray(
        W[:, :E].T.reshape(ET, P, D).transpose(1, 0, 2).reshape(P, ET * D)
    )
    z_all = dec_state @ W[:, E:].T + np.asarray(W_bias, dtype=np.float32)  # [B, D]
    vrep_h = np.ascontiguousarray(
        np.broadcast_to(np.asarray(v_weight, dtype=np.float32).reshape(1, D), (P, D))
    )
    ident_h = np.eye(P, dtype=np.float32)
    onesc_h = np.ones((P, 1), dtype=np.float32)

    in_maps = []
    for c in range(CORES):
        zrep_h = np.ascontiguousarray(
            np.broadcast_to(z_all[None, c * bl : (c + 1) * bl, :], (P, bl, D))
        )
        in_maps.append(
            {
                "enc": enc_out[c * bl : (c + 1) * bl],
                "wet": wet_h,
                "zrepp": zrep_h,
                "vrep": vrep_h,
                "ident": ident_h,
                "onesc": onesc_h,
            }
        )
    return in_maps


_NC_CACHE = {}


def _get_nc():
    if "nc" not in _NC_CACHE:
        _NC_CACHE["nc"] = _build_kernel()
    return _NC_CACHE["nc"]


def _run(inputs, trace=False, tmpdir=None):
    nc = _get_nc()
    in_maps = _prep_inputs(
        inputs["enc_out"],
        inputs["dec_state"],
        inputs["W_weight"],
        inputs["W_bias"],
        inputs["v_weight"],
    )
    res = run_bass_kernel_spmd(
        nc, in_maps, list(range(CORES)), trace=trace, tmpdir=tmpdir
    )
    out = np.concatenate(
        [np.asarray(res.results[c]["ctx_out"]) for c in range(CORES)], axis=0
    )
    return out.astype(np.float32, copy=False), res


def kernel(**inputs):
    out, _ = _run(inputs, trace=False)
    return out



# revision 6
# speedup vs baseline: 2.2364x; 2.2364x over previous
"""Trainium2 Bass kernel for additive (Bahdanau) attention — fp8 DoubleRow version.

  context[b] = sum_t softmax_t( v . tanh(We @ enc[b,t] + Wd @ dec[b] + bias) ) * enc[b,t]

Shapes (hardcoded): enc_out [64, 2048, 1024] f32, dec_state [64, 1024] f32,
W_weight [1024, 2048], W_bias [1024], v_weight [1, 1024].  Output [64, 1024].

Sharding: data-parallel over batch across 8 NeuronCores (8 batches/core).

Design (per core, per batch-half of TH=1024 timesteps):
  - Host pre-transposes enc to XT tiles [e_loc(128 part), e_tile(8), t] in bf16
    (16KB/partition contiguous DMA) plus an fp8(e4m3) copy of the first KF8
    e-tiles; the rest are cast bf16->fp8 on DVE.  No PE transpose at all.
  - projT[d,t] = 64*(We @ X^T) via fp8 DoubleRow matmuls (0.5 cyc/row, K=256
    per instr), weights stationary (wetp = We^T * 64 as [128,2,128] pairs).
  - ACT: energy = tanh(projT * 1/64 + z) fused via per-partition bias
    (z = Wd @ dec + W_bias host-computed), output fp8.
  - scores: v-dot as fp8 DoubleRow matmul with v*64 replicated across 128
    output columns -> score rows arrive replicated on all partitions.
  - ACT: p = exp(score * 1/64) -> bf16, with free accum_out Sigma(p).
  - ctx: DVE scalar_tensor_tensor per e-tile: accum_out[e_loc] = sum_t
    XTbf16[e,t] * p[t] (all-bf16 operands -> DVE 2x/4x mode), f32 accum.
  - batch end: combine halves, reciprocal, scale, tiny PE transpose
    [128,8]->[8,128], DMA out.

Numerics (numpy sim): rel err ~1.5e-2 vs f64 reference (gate 2e-2); dominated
by e4m3 quantization of X and We in the 87.5%-of-FLOPs projection matmul.
"""

import sys

sys.path.insert(0, "/opt/trn_rl_repo")

from contextlib import ExitStack

import ml_dtypes
import numpy as np

import concourse.tile as tile
from concourse import bacc, mybir
from concourse.bass_utils import run_bass_kernel_spmd

F32 = mybir.dt.float32
F8 = mybir.dt.float8e4
BF16 = mybir.dt.bfloat16
DR = mybir.MatmulPerfMode.DoubleRow
NPF8 = ml_dtypes.float8_e4m3fn
NPBF = ml_dtypes.bfloat16

B, T, E, D = 64, 2048, 1024, 1024
CORES = 8
BL = B // CORES      # batches per core
P = 128              # partitions
TH = 1024            # timesteps per half-batch
NH = T // TH         # halves per batch (2)
NJ = E // P          # e-tiles / d-tiles (8)
NK = NJ // 2         # DoubleRow e-pairs (4)
NI = TH // 256       # moving quarters per half (4)
KF8 = 4              # e-tiles whose fp8 copy comes from DMA; rest DVE-cast
WSCALE = 64.0        # fp8 exponent headroom for We and v entries


def _build_kernel():
    nc = bacc.Bacc(
        "TRN2",
        target_bir_lowering=False,
        debug=False,
        num_devices=CORES,
    )

    xtb = nc.declare_dram_parameter("xtb", [BL, NH, P, NJ, TH], BF16, isOutput=False)
    xt8 = nc.declare_dram_parameter("xt8", [BL, NH, P, KF8, TH], F8, isOutput=False)
    wetp = nc.declare_dram_parameter("wetp", [P, NK, 2, D], F8, isOutput=False)
    vrp = nc.declare_dram_parameter("vrp", [P, NK, 2, P], F8, isOutput=False)
    zc = nc.declare_dram_parameter("zc", [P, BL * NJ], F32, isOutput=False)
    ident = nc.declare_dram_parameter("ident", [P, P], F32, isOutput=False)
    out = nc.declare_dram_parameter("ctx_out", [BL, E], F32, isOutput=True)

    with tile.TileContext(nc) as tc, ExitStack() as ctx:
        const = ctx.enter_context(tc.tile_pool(name="const", bufs=1))
        xbpool = ctx.enter_context(tc.tile_pool(name="xb", bufs=3))
        x8pool = ctx.enter_context(tc.tile_pool(name="x8", bufs=3))
        epool = ctx.enter_context(tc.tile_pool(name="en", bufs=2))
        ppool = ctx.enter_context(tc.tile_pool(name="p", bufs=2))
        spool = ctx.enter_context(tc.tile_pool(name="scr", bufs=2))
        small = ctx.enter_context(tc.tile_pool(name="small", bufs=2))

        ps_proj = ctx.enter_context(tc.tile_pool(name="ps_proj", bufs=2, space="PSUM"))
        ps_score = ctx.enter_context(tc.tile_pool(name="ps_score", bufs=1, space="PSUM"))
        ps_misc = ctx.enter_context(tc.tile_pool(name="ps_misc", bufs=1, space="PSUM"))

        # ---- resident constants; first step's tiles go first on the queue
        xb_pre = xbpool.tile([P, NJ, TH], BF16, tag="xb", name="xb_pre")
        nc.sync.dma_start(xb_pre[:], xtb[0, 0])
        x8_pre = x8pool.tile([P, NJ, TH], F8, tag="x8", name="x8_pre")
        if KF8:
            nc.sync.dma_start(x8_pre[:, 0:KF8, :], xt8[0, 0])
        wetp_sb = const.tile([P, NK, 2, D], F8)
        nc.sync.dma_start(wetp_sb[:], wetp[:])
        vrp_sb = const.tile([P, NK, 2, P], F8)
        nc.sync.dma_start(vrp_sb[:], vrp[:])
        zc_sb = const.tile([P, BL * NJ], F32)
        nc.sync.dma_start(zc_sb[:], zc[:])
        ident_sb = const.tile([P, P], F32)
        nc.sync.dma_start(ident_sb[:], ident[:])

        state = {}

        def get_bstate(b):
            if b not in state:
                state[b] = dict(
                    ctxc=small.tile([P, NH * NJ], F32, tag="ctxc", name=f"ctxc{b}"),
                    lcol=small.tile([P, NH], F32, tag="lcol", name=f"lcol{b}"),
                )
            return state[b]

        def emit_half(b, h, xb_t, x8_t):
            st = get_bstate(b)
            # fp8 tiles not provided by DMA: cast on DVE from the bf16 copy
            for j in range(KF8, NJ):
                nc.vector.tensor_copy(x8_t[:, j, :], xb_t[:, j, :])

            e_t = epool.tile([P, NJ, TH], F8, tag="en")
            score_ps = ps_score.tile([P, TH], F32, tag="score")
            pj_list = [None] * NJ

            def emit_vdot(dp):
                # NOTE: 512-wide moving chunks; 256-wide chunks with reused
                # DoubleRow weights drop the k=0 term on alternating regions
                # (hw erratum, see probe4).
                rhs3 = e_t[:, 2 * dp : 2 * dp + 2, :]
                for i in range(2):
                    nc.tensor.matmul(
                        score_ps[:, i * 512 : (i + 1) * 512],
                        vrp_sb[:, dp],
                        rhs3[:, :, i * 512 : (i + 1) * 512],
                        start=(dp == 0),
                        stop=(dp == NK - 1),
                        perf_mode=DR,
                    )

            for j in range(NJ):
                pj = ps_proj.tile([P, TH], F32, tag="proj")
                pj_list[j] = pj
                for k in range(NK):
                    lhsT = wetp_sb[:, k, :, j * P : (j + 1) * P]
                    rhs3 = x8_t[:, 2 * k : 2 * k + 2, :]
                    for i in range(2):
                        nc.tensor.matmul(
                            pj[:, i * 512 : (i + 1) * 512],
                            lhsT,
                            rhs3[:, :, i * 512 : (i + 1) * 512],
                            start=(k == 0),
                            stop=(k == NK - 1),
                            perf_mode=DR,
                        )
                # energy_j = tanh(proj/WSCALE + z[b, j])  -> fp8
                nc.scalar.activation(
                    e_t[:, j, :],
                    pj[:],
                    mybir.ActivationFunctionType.Tanh,
                    bias=zc_sb[:, b * NJ + j : b * NJ + j + 1],
                    scale=1.0 / WSCALE,
                )
                # lag the score matmuls two j's behind tanh to keep the
                # in-order PE queue from stalling on ACT
                if j >= 3 and j % 2 == 1:
                    emit_vdot((j - 3) // 2)
            emit_vdot(NK - 1)

            # p = exp(score/WSCALE) -> bf16 (replicated rows);  l = sum_t p
            p_t = ppool.tile([P, TH], BF16, tag="p")
            nc.scalar.activation(
                p_t[:],
                score_ps[:],
                mybir.ActivationFunctionType.Exp,
                scale=1.0 / WSCALE,
                accum_out=st["lcol"][:, h : h + 1],
            )

            # ctx_half[e] += sum_t XT[e, t] * p[t]   (DVE, f32 accum)
            scr = spool.tile([P, TH], BF16, tag="scr")
            for j in range(NJ):
                nc.vector.scalar_tensor_tensor(
                    out=scr[:],
                    in0=xb_t[:, j, :],
                    scalar=1.0,
                    in1=p_t[:],
                    op0=mybir.AluOpType.mult,
                    op1=mybir.AluOpType.mult,
                    accum_out=st["ctxc"][:, h * NJ + j : h * NJ + j + 1],
                )

            if h == NH - 1:
                emit_batch_end(b)

        def emit_batch_end(b):
            st = state.pop(b)
            ctx8 = small.tile([P, NJ], F32, tag="ctx8")
            nc.vector.tensor_add(
                ctx8[:], st["ctxc"][:, 0:NJ], st["ctxc"][:, NJ : 2 * NJ]
            )
            lsum = small.tile([P, 1], F32, tag="lsum")
            nc.vector.tensor_add(
                lsum[:], st["lcol"][:, 0:1], st["lcol"][:, 1:2]
            )
            linv = small.tile([P, 1], F32, tag="linv")
            nc.vector.reciprocal(linv[:], lsum[:])
            ctx8s = small.tile([P, NJ], F32, tag="ctx8s")
            nc.scalar.activation(
                ctx8s[:], ctx8[:],
                mybir.ActivationFunctionType.Copy, scale=linv[:],
            )
            ctp = ps_misc.tile([NJ, P], F32, tag="ctp")
            nc.tensor.transpose(ctp[:], ctx8s[:], ident_sb[:])
            ctxrow = small.tile([NJ, P], F32, tag="ctxrow")
            nc.scalar.copy(ctxrow[:], ctp[:])
            nc.sync.dma_start(out[b : b + 1, :], ctxrow[:])

        for step in range(BL * NH):
            b, h = divmod(step, NH)
            if step == 0:
                xb_t, x8_t = xb_pre, x8_pre
            else:
                xb_t = xbpool.tile([P, NJ, TH], BF16, tag="xb")
                nc.sync.dma_start(xb_t[:], xtb[b, h])
                x8_t = x8pool.tile([P, NJ, TH], F8, tag="x8")
                if KF8:
                    nc.sync.dma_start(x8_t[:, 0:KF8, :], xt8[b, h])
            emit_half(b, h, xb_t, x8_t)

    nc.compile()
    return nc


def _prep_inputs(enc_out, dec_state, W_weight, W_bias, v_weight):
    """Host-side layout prep: per-core transposes to [e_loc, e_tile, t] tiles,
    fp8 casts with x64 weight scaling, and the tiny z = Wd@dec + bias term
    (0.05% of FLOPs)."""
    W = np.asarray(W_weight, dtype=np.float32)
    We = W[:, :E]
    z_all = (
        np.asarray(dec_state, dtype=np.float32) @ W[:, E:].T
        + np.asarray(W_bias, dtype=np.float32)
    )  # [B, D]

    # wetp[p, k, i, d] = We[d, (2k+i)*128 + p] * WSCALE
    wetp_h = np.ascontiguousarray(
        (We.T * WSCALE).reshape(NK, 2, P, D).transpose(2, 0, 1, 3)
    ).astype(NPF8)
    # vrp[p, dp, i, m] = v[(2dp+i)*128 + p] * WSCALE  (replicated over m)
    v64 = (np.asarray(v_weight, dtype=np.float32).reshape(D) * WSCALE).reshape(
        NK, 2, P
    )
    vrp_h = np.ascontiguousarray(
        np.broadcast_to(v64.transpose(2, 0, 1)[:, :, :, None], (P, NK, 2, P))
    ).astype(NPF8)
    ident_h = np.eye(P, dtype=np.float32)

    enc_out = np.asarray(enc_out, dtype=np.float32)
    in_maps = []
    for c in range(CORES):
        encc = enc_out[c * BL : (c + 1) * BL]
        # [b, h, t, j, p] -> [b, h, p, j, t]
        xtb_h = np.ascontiguousarray(
            encc.astype(NPBF).reshape(BL, NH, TH, NJ, P).transpose(0, 1, 4, 3, 2)
        )
        xt8_h = np.ascontiguousarray(xtb_h[:, :, :, :KF8, :]).astype(NPF8)
        zc_h = np.ascontiguousarray(
            z_all[c * BL : (c + 1) * BL].reshape(BL, NJ, P).transpose(2, 0, 1)
        ).reshape(P, BL * NJ)
        in_maps.append(
            {
                "xtb": xtb_h,
                "xt8": xt8_h,
                "wetp": wetp_h,
                "vrp": vrp_h,
                "zc": zc_h,
                "ident": ident_h,
            }
        )
    return in_maps


_NC_CACHE = {}


def _get_nc():
    if "nc" not in _NC_CACHE:
        _NC_CACHE["nc"] = _build_kernel()
    return _NC_CACHE["nc"]


def _run(inputs, trace=False, tmpdir=None):
    nc = _get_nc()
    in_maps = _prep_inputs(
        inputs["enc_out"],
        inputs["dec_state"],
        inputs["W_weight"],
        inputs["W_bias"],
        inputs["v_weight"],
    )
    res = run_bass_kernel_spmd(
        nc, in_maps, list(range(CORES)), trace=trace, tmpdir=tmpdir
    )
    out = np.concatenate(
        [np.asarray(res.results[c]["ctx_out"]) for c in range(CORES)], axis=0
    )
    return out.astype(np.float32, copy=False), res


def kernel(**inputs):
    out, _ = _run(inputs, trace=False)
    return out


# revision 14
# speedup vs baseline: 2.5671x; 1.1479x over previous
"""Trainium2 Bass kernel for additive (Bahdanau) attention — fp8 DoubleRow version.

  context[b] = sum_t softmax_t( v . tanh(We @ enc[b,t] + Wd @ dec[b] + bias) ) * enc[b,t]

Shapes (hardcoded): enc_out [64, 2048, 1024] f32, dec_state [64, 1024] f32,
W_weight [1024, 2048], W_bias [1024], v_weight [1, 1024].  Output [64, 1024].

Sharding: data-parallel over batch across 8 NeuronCores (8 batches/core).

Design (per core, per batch-half of TH=1024 timesteps):
  - Host pre-transposes enc to XT tiles [e_loc(128 part), e_tile(8), t] in bf16
    (16KB/partition contiguous DMA) plus an fp8(e4m3) copy of the first KF8
    e-tiles; the rest are cast bf16->fp8 on DVE.  No PE transpose at all.
  - projT[d,t] = 64*(We @ X^T) via fp8 DoubleRow matmuls (0.5 cyc/row, K=256
    per instr), weights stationary (wetp = We^T * 64 as [128,2,128] pairs).
  - ACT: energy = tanh(projT * 1/64 + z) fused via per-partition bias
    (z = Wd @ dec + W_bias host-computed), output fp8.
  - scores: v-dot as fp8 DoubleRow matmul with v*64 replicated across 128
    output columns -> score rows arrive replicated on all partitions.
  - ACT: p = exp(score * 1/64) -> bf16, with free accum_out Sigma(p).
  - ctx: DVE scalar_tensor_tensor per e-tile: accum_out[e_loc] = sum_t
    XTbf16[e,t] * p[t] (all-bf16 operands -> DVE 2x/4x mode), f32 accum.
  - batch end: combine halves, reciprocal, scale, tiny PE transpose
    [128,8]->[8,128], DMA out.

Numerics (numpy sim): rel err ~1.5e-2 vs f64 reference (gate 2e-2); dominated
by e4m3 quantization of X and We in the 87.5%-of-FLOPs projection matmul.
"""

import sys

sys.path.insert(0, "/opt/trn_rl_repo")

from contextlib import ExitStack

import ml_dtypes
import numpy as np

import concourse.tile as tile
from concourse import bacc, mybir
from concourse.bass_utils import run_bass_kernel_spmd

F32 = mybir.dt.float32
F8 = mybir.dt.float8e4
BF16 = mybir.dt.bfloat16
DR = mybir.MatmulPerfMode.DoubleRow
DRS = mybir.MatmulPerfMode.DoubleRowSwInterleave
NPF8 = ml_dtypes.float8_e4m3fn
NPBF = ml_dtypes.bfloat16

B, T, E, D = 64, 2048, 1024, 1024
CORES = 8
BL = B // CORES      # batches per core
P = 128              # partitions
TH = 1024            # timesteps per half-batch
NH = T // TH         # halves per batch (2)
NJ = E // P          # e-tiles / d-tiles (8)
NK = NJ // 2         # DoubleRow e-pairs (4)
NI = TH // 256       # moving quarters per half (4)
KF8 = 4              # e-tiles whose fp8 copy comes from DMA; rest DVE-cast
WSCALE = 64.0        # fp8 exponent headroom for We and v entries


def _build_kernel():
    nc = bacc.Bacc(
        "TRN2",
        target_bir_lowering=False,
        debug=False,
        num_devices=CORES,
    )

    xtb = nc.declare_dram_parameter("xtb", [BL, NH, P, NJ, TH], BF16, isOutput=False)
    xt8 = nc.declare_dram_parameter("xt8", [BL, NH, P, KF8, TH], F8, isOutput=False)
    # SwInterleave layouts: per weight block, 256 cols c=2m+s hold
    # slot_s[:, 127-m] (see bass_interp DoubleRowSwInterleave)
    wetp = nc.declare_dram_parameter("wetp", [P, NK, NJ, 2, P], F8, isOutput=False)
    vrp = nc.declare_dram_parameter("vrp", [P, NK, 2, P], F8, isOutput=False)
    zc = nc.declare_dram_parameter("zc", [P, BL * NJ], F32, isOutput=False)
    ident = nc.declare_dram_parameter("ident", [P, P], F32, isOutput=False)
    out = nc.declare_dram_parameter("ctx_out", [BL, E], F32, isOutput=True)

    with tile.TileContext(nc) as tc, ExitStack() as ctx:
        const = ctx.enter_context(tc.tile_pool(name="const", bufs=1))
        xbpool = ctx.enter_context(tc.tile_pool(name="xb", bufs=3))
        x8pool = ctx.enter_context(tc.tile_pool(name="x8", bufs=3))
        epool = ctx.enter_context(tc.tile_pool(name="en", bufs=2))
        ppool = ctx.enter_context(tc.tile_pool(name="p", bufs=2))
        spool = ctx.enter_context(tc.tile_pool(name="scr", bufs=2))
        small = ctx.enter_context(tc.tile_pool(name="small", bufs=2))

        ps_proj = ctx.enter_context(tc.tile_pool(name="ps_proj", bufs=2, space="PSUM"))
        ps_score = ctx.enter_context(tc.tile_pool(name="ps_score", bufs=1, space="PSUM"))
        ps_misc = ctx.enter_context(tc.tile_pool(name="ps_misc", bufs=1, space="PSUM"))

        # ---- resident constants; first step's tiles go first on the queue
        xb_pre = xbpool.tile([P, NJ, TH], BF16, tag="xb", name="xb_pre")
        nc.sync.dma_start(xb_pre[:], xtb[0, 0])
        x8_pre = x8pool.tile([P, NJ, TH], F8, tag="x8", name="x8_pre")
        if KF8:
            nc.sync.dma_start(x8_pre[:, 0:KF8, :], xt8[0, 0])
        wetp_sb = const.tile([P, NK, NJ, 2, P], F8)
        nc.sync.dma_start(wetp_sb[:], wetp[:])
        vrp_sb = const.tile([P, NK, 2, P], F8)
        nc.sync.dma_start(vrp_sb[:], vrp[:])
        zc_sb = const.tile([P, BL * NJ], F32)
        nc.sync.dma_start(zc_sb[:], zc[:])
        ident_sb = const.tile([P, P], F32)
        nc.sync.dma_start(ident_sb[:], ident[:])

        state = {}

        def get_bstate(b):
            if b not in state:
                state[b] = dict(
                    ctxc=small.tile([P, NH * NJ], F32, tag="ctxc", name=f"ctxc{b}"),
                    lcol=small.tile([P, NH], F32, tag="lcol", name=f"lcol{b}"),
                )
            return state[b]

        def emit_half(b, h, xb_t, x8_t):
            st = get_bstate(b)
            e_t = epool.tile([P, NJ, TH], F8, tag="en")
            score_ps = ps_score.tile([P, TH], F32, tag="score")
            pj_list = [None] * NJ

            def emit_vdot(dp):
                # NOTE: 512-wide moving chunks; 256-wide chunks with reused
                # DoubleRow weights drop the k=0 term on alternating regions
                # (hw erratum, see probe4).
                rhs3 = e_t[:, 2 * dp : 2 * dp + 2, :]
                for i in range(2):
                    nc.tensor.matmul(
                        score_ps[:, i * 512 : (i + 1) * 512],
                        vrp_sb[:, dp],
                        rhs3[:, :, i * 512 : (i + 1) * 512],
                        start=(dp == 0),
                        stop=(dp == NK - 1),
                        perf_mode=DRS,
                    )

            for j in range(NJ):
                pj = ps_proj.tile([P, TH], F32, tag="proj")
                pj_list[j] = pj
                for k in range(NK):
                    lhsT = wetp_sb[:, k, j]
                    rhs3 = x8_t[:, 2 * k : 2 * k + 2, :]
                    for i in range(2):
                        nc.tensor.matmul(
                            pj[:, i * 512 : (i + 1) * 512],
                            lhsT,
                            rhs3[:, :, i * 512 : (i + 1) * 512],
                            start=(k == 0),
                            stop=(k == NK - 1),
                            perf_mode=DRS,
                        )
                # energy_j = tanh(proj/WSCALE + z[b, j])  -> fp8
                nc.scalar.activation(
                    e_t[:, j, :],
                    pj[:],
                    mybir.ActivationFunctionType.Tanh,
                    bias=zc_sb[:, b * NJ + j : b * NJ + j + 1],
                    scale=1.0 / WSCALE,
                )
                # lag the score matmuls two j's behind tanh to keep the
                # in-order PE queue from stalling on ACT
                if j >= 3 and j % 2 == 1:
                    emit_vdot((j - 3) // 2)
            emit_vdot(NK - 1)

            # p = exp(score/WSCALE) -> bf16 (replicated rows);  l = sum_t p
            p_t = ppool.tile([P, TH], BF16, tag="p")
            nc.scalar.activation(
                p_t[:],
                score_ps[:],
                mybir.ActivationFunctionType.Exp,
                scale=1.0 / WSCALE,
                accum_out=st["lcol"][:, h : h + 1],
            )

            # ctx_half[e] += sum_t XT[e, t] * p[t]   (DVE, f32 accum)
            scr = spool.tile([P, TH], BF16, tag="scr")
            for j in range(NJ):
                nc.vector.scalar_tensor_tensor(
                    out=scr[:],
                    in0=xb_t[:, j, :],
                    scalar=1.0,
                    in1=p_t[:],
                    op0=mybir.AluOpType.mult,
                    op1=mybir.AluOpType.mult,
                    accum_out=st["ctxc"][:, h * NJ + j : h * NJ + j + 1],
                )

            if h == NH - 1:
                emit_batch_end(b)

        def emit_batch_end(b):
            st = state.pop(b)
            ctx8 = small.tile([P, NJ], F32, tag="ctx8")
            nc.vector.tensor_add(
                ctx8[:], st["ctxc"][:, 0:NJ], st["ctxc"][:, NJ : 2 * NJ]
            )
            lsum = small.tile([P, 1], F32, tag="lsum")
            nc.vector.tensor_add(
                lsum[:], st["lcol"][:, 0:1], st["lcol"][:, 1:2]
            )
            linv = small.tile([P, 1], F32, tag="linv")
            nc.vector.reciprocal(linv[:], lsum[:])
            ctx8s = small.tile([P, NJ], F32, tag="ctx8s")
            nc.scalar.activation(
                ctx8s[:], ctx8[:],
                mybir.ActivationFunctionType.Copy, scale=linv[:],
            )
            ctp = ps_misc.tile([NJ, P], F32, tag="ctp")
            nc.tensor.transpose(ctp[:], ctx8s[:], ident_sb[:])
            ctxrow = small.tile([NJ, P], F32, tag="ctxrow")
            nc.scalar.copy(ctxrow[:], ctp[:])
            nc.sync.dma_start(out[b : b + 1, :], ctxrow[:])

        # prefetch one step ahead: DMA + DVE fp8 casts for step s+1 are queued
        # before step s's compute so the PE never waits at half boundaries
        tiles = {}

        def fetch(step):
            b, h = divmod(step, NH)
            if step == 0:
                xb_t, x8_t = xb_pre, x8_pre
            else:
                xb_t = xbpool.tile([P, NJ, TH], BF16, tag="xb")
                nc.sync.dma_start(xb_t[:], xtb[b, h])
                x8_t = x8pool.tile([P, NJ, TH], F8, tag="x8")
                if KF8:
                    nc.sync.dma_start(x8_t[:, 0:KF8, :], xt8[b, h])
            for j in range(KF8, NJ):
                nc.vector.tensor_copy(x8_t[:, j, :], xb_t[:, j, :])
            tiles[step] = (xb_t, x8_t)

        total = BL * NH
        fetch(0)
        for step in range(total):
            if step + 1 < total:
                fetch(step + 1)
            b, h = divmod(step, NH)
            emit_half(b, h, *tiles.pop(step))

    nc.compile()
    return nc


def _prep_inputs(enc_out, dec_state, W_weight, W_bias, v_weight):
    """Host-side layout prep: per-core transposes to [e_loc, e_tile, t] tiles,
    fp8 casts with x64 weight scaling, and the tiny z = Wd@dec + bias term
    (0.05% of FLOPs)."""
    W = np.asarray(W_weight, dtype=np.float32)
    We = W[:, :E]
    z_all = (
        np.asarray(dec_state, dtype=np.float32) @ W[:, E:].T
        + np.asarray(W_bias, dtype=np.float32)
    )  # [B, D]

    # SwInterleave weight blocks: block (k, j) col c=2m+s holds
    # We[(j*128 + 127 - m), (2k+s)*128 + p] * WSCALE
    arr = (We.T * WSCALE).reshape(NK, 2, P, NJ, P)  # [k, s, p(e_loc), j, dl]
    a2 = arr[:, :, :, :, ::-1]                      # dl -> m reversed
    wetp_h = np.ascontiguousarray(a2.transpose(2, 0, 3, 4, 1)).reshape(
        P, NK, NJ, 2, P
    ).astype(NPF8)
    # vrp block (dp): col c=2m+s holds v[(2dp+s)*128 + p] * WSCALE (any m)
    v64 = (np.asarray(v_weight, dtype=np.float32).reshape(D) * WSCALE).reshape(
        NK, 2, P
    )
    vs = v64.transpose(2, 0, 1)                     # [p, dp, s]
    vrp_h = np.ascontiguousarray(
        np.broadcast_to(vs[:, :, None, :], (P, NK, P, 2)).reshape(P, NK, 2, P)
    ).astype(NPF8)
    ident_h = np.eye(P, dtype=np.float32)

    enc_out = np.asarray(enc_out, dtype=np.float32)
    in_maps = []
    for c in range(CORES):
        encc = enc_out[c * BL : (c + 1) * BL]
        # [b, h, t, j, p] -> [b, h, p, j, t]
        xtb_h = np.ascontiguousarray(
            encc.astype(NPBF).reshape(BL, NH, TH, NJ, P).transpose(0, 1, 4, 3, 2)
        )
        xt8_h = np.ascontiguousarray(xtb_h[:, :, :, :KF8, :]).astype(NPF8)
        zc_h = np.ascontiguousarray(
            z_all[c * BL : (c + 1) * BL].reshape(BL, NJ, P).transpose(2, 0, 1)
        ).reshape(P, BL * NJ)
        in_maps.append(
            {
                "xtb": xtb_h,
                "xt8": xt8_h,
                "wetp": wetp_h,
                "vrp": vrp_h,
                "zc": zc_h,
                "ident": ident_h,
            }
        )
    return in_maps


_NC_CACHE = {}


def _get_nc():
    if "nc" not in _NC_CACHE:
        _NC_CACHE["nc"] = _build_kernel()
    return _NC_CACHE["nc"]


def _run(inputs, trace=False, tmpdir=None):
    nc = _get_nc()
    in_maps = _prep_inputs(
        inputs["enc_out"],
        inputs["dec_state"],
        inputs["W_weight"],
        inputs["W_bias"],
        inputs["v_weight"],
    )
    res = run_bass_kernel_spmd(
        nc, in_maps, list(range(CORES)), trace=trace, tmpdir=tmpdir
    )
    out = np.concatenate(
        [np.asarray(res.results[c]["ctx_out"]) for c in range(CORES)], axis=0
    )
    return out.astype(np.float32, copy=False), res


def kernel(**inputs):
    out, _ = _run(inputs, trace=False)
    return out
